# revision 38
# baseline (speedup 1.0000x reference)
"""BitConv2dInfer on 8 Trainium2 NeuronCores — fp8 DoubleRow, host-padded.

Reference computation (per full input):
    x = clip(x, -1, 1)                       # x [32, 256, 56, 56] f32
    y = conv2d(x, w_q, pad=1)                # w_q [256, 256, 3, 3] ternary
    y = y * s + bias                         # per-out-channel affine
Sharding: data-parallel over batch — each of the 8 cores gets 4 images and
the full (tiny) weights; outputs concatenate over batch with no comms.

Numerics: the clamp and the e4m3 quantization both happen on the HOST
(f32 -> clip -> e4m3), so the device sees conv-ready fp8 planes. Ternary
weights are exact in e4m3. The PE runs e4m3 DoubleRow matmuls (virtual
K=256, 2 fp8 mults/cell/cycle): 189.6 ns per 448-column matmul measured —
the DR roofline — and the steady-state stream is gapless.

Host also pre-pads: x ships as [NB, NCI, 128, 58, 64] zero-bordered planes
(64-wide rows keep the ci-plane stride 16B-aligned for the DoubleRow
ifmap AP rules; cols 58-63 never read). This kills the on-device clamp
and border memsets entirely and halves input HBM bytes vs bf16 transport.
The head of the kernel is ring-bandwidth-bound (~50-70 B/ns per ring
while all rings are hot, ~0.7us of descriptor-issue time per transfer),
so the first-matmul critical path carries only the head rows and the co0
weights. (Nibble-packing the ternary weights was tried and LOSES: the
DVE unpack latency on the critical path exceeds the ring-time saved.)

Device kernel (per core, per image):
  - image 0: row-chunked DMAs straight into the pad tile in deadline
    order on the two HWDGE rings (sync: ci0 rows + packed weights,
    scalar: ci1 rows; the slower SWDGE gpsimd ring gets only sb and the
    bottom rows); image 0 leads with four co0 groups so the co1 weights
    get +6.8us of slack
  - image 1 loads in row chunks on the scalar ring (DMA-lane reuse
    paces the chunks so they cannot crowd out the head-critical
    transfers); images 2-3 load whole planes on gpsimd, paced by the
    bufs=2 pad-tile pool (their transfers wait for image n-2's last
    read). The idle vector engine builds the 1-col-shifted copy B
    (even-aligned kw=1 tap) for images 1-3.
  - conv as 9 accumulated DoubleRow PE matmuls per (cout_tile, 8-row
    chunk): 189.6 ns each, stream measured gapless end to end
  - scalar-engine activation evacuates PSUM with per-partition scale+bias
  - images 0-2 DMA bf16 result tiles out over sync+scalar after each
    image; the last image streams each group as its ACT lands, rotating
    sync/gpsimd/scalar with the groups just before the final one forced
    to gpsimd, and the final group's transfer splits scalar+sync so the
    tail drains with no ring backlog

The PE clock gate (HAM) starts at 1.2 GHz and needs ~3.4us of sustained
activity to reach 2.4 GHz, so the kernel front-runs dummy matmuls on a
zeroed tile while the first input chunks are in flight.
"""

import sys

sys.path.insert(0, "/opt/trn_rl_repo")

import ml_dtypes  # noqa: F401
import numpy as np

import concourse.bass as bass  # noqa: F401  (registers engines)
import concourse.mybir as mybir
import concourse.tile as tile
from concourse import bacc
from concourse.bass_utils import run_bass_kernel_spmd

N, CIN, COUT, H, W = 32, 256, 256, 56, 56
NCORES = 8
NB = N // NCORES          # images per core
HP = H + 2                # padded rows
WPP = 64                  # padded row pitch (56+2 used; 64 keeps plane 16B-aligned)
RG = 8                    # output rows per PSUM chunk (8*56=448 <= 512 f32/bank)
NCH = H // RG             # chunks per image
NCI = CIN // 128          # cin tiles
NCO = COUT // 128         # cout tiles
NTAP = 9
N_WARM_MM = 10            # dummy matmuls to lift the HAM clock gate

_compiled = {}


def _build():
    nc = bacc.Bacc("TRN2", target_bir_lowering=False, debug=False)
    f32, bf16, fp8 = mybir.dt.float32, mybir.dt.bfloat16, mybir.dt.float8e4
    DR = mybir.MatmulPerfMode.DoubleRow
    x_d = nc.dram_tensor(
        "x", [NB, NCI, 128, HP, WPP], fp8, kind="ExternalInput"
    ).ap()
    w_d = nc.dram_tensor(
        "w", [128, NCO, NTAP, NCI, 128], fp8, kind="ExternalInput"
    ).ap()
    sb_d = nc.dram_tensor("sb", [128, 2 * NCO], f32, kind="ExternalInput").ap()
    o_d = nc.dram_tensor("out", [NB, COUT, H, W], bf16, kind="ExternalOutput").ap()

    # values are already in [-1,1], so the clamp is numerically a copy
    copy_ops = dict(op0=mybir.AluOpType.max, op1=mybir.AluOpType.min)

    with tile.TileContext(nc) as tc:
        with (
            tc.tile_pool(name="const", bufs=1) as cpool,
            # bufs=2 doubles as prefetch pacing: image n+2's DMA reuses
            # image n's buffer, so its transfer can't start (and steal
            # head/steady bandwidth) until image n's last matmul read.
            tc.tile_pool(name="xpad", bufs=2) as xppool,
            tc.tile_pool(name="xpadb", bufs=2) as xpbpool,
            tc.tile_pool(name="osb", bufs=3) as opool,
            tc.tile_pool(name="ps", bufs=6, space="PSUM") as pspool,
            tc.tile_pool(name="warmps", bufs=1, space="PSUM") as wpspool,
        ):
            w_sb = cpool.tile([128, NCO, NTAP, NCI, 128], fp8, tag="w")
            sb_sb = cpool.tile([128, 2 * NCO], f32, tag="sb")

            # HAM pre-warm: the memset rides the gpsimd queue, which clears
            # its preamble first, so the warm matmuls start the PE clock
            # ramp ~1us sooner than a vector-side memset would allow.
            # The product lands in a PSUM bank nobody reads.
            warm = cpool.tile([128, RG * W], fp8, tag="warm")
            nc.gpsimd.memset(warm[:], 0.0)
            warm_ps = wpspool.tile([128, RG * W], f32, tag="warmps")
            for _ in range(N_WARM_MM):
                nc.tensor.matmul(
                    out=warm_ps[:], lhsT=warm[:, 0:128], rhs=warm[:],
                    start=True, stop=True,
                )

            # First image: everything on the first-matmul critical path
            # rides the two HWDGE rings in deadline order (the SWDGE
            # gpsimd ring both issues and transfers much slower while the
            # HW rings are busy — it gets only sb and the bottom rows,
            # which aren't needed until ~28us). Weights split across both
            # rings; co1 has +6.8us of slack because image 0 leads with
            # four co0 groups.
            xp0 = xppool.tile([128, NCI, HP, WPP], fp8, tag="xpad")
            nc.sync.dma_start(out=xp0[:, 0, 0:10], in_=x_d[0, 0, :, 0:10])
            nc.scalar.dma_start(out=xp0[:, 1, 0:10], in_=x_d[0, 1, :, 0:10])
            nc.sync.dma_start(out=w_sb[:, 0, 0:5], in_=w_d[:, 0, 0:5])
            nc.scalar.dma_start(out=w_sb[:, 0, 5:9], in_=w_d[:, 0, 5:9])
            nc.gpsimd.dma_start(out=sb_sb[:], in_=sb_d)
            nc.sync.dma_start(out=xp0[:, 0, 10:18], in_=x_d[0, 0, :, 10:18])
            nc.scalar.dma_start(out=xp0[:, 1, 10:26], in_=x_d[0, 1, :, 10:26])
            nc.sync.dma_start(out=xp0[:, 0, 18:26], in_=x_d[0, 0, :, 18:26])
            nc.gpsimd.dma_start(out=xp0[:, 0, 26:42], in_=x_d[0, 0, :, 26:42])
            nc.scalar.dma_start(out=xp0[:, 1, 26:42], in_=x_d[0, 1, :, 26:42])
            nc.sync.dma_start(out=w_sb[:, 1, 0:5], in_=w_d[:, 1, 0:5])
            nc.scalar.dma_start(out=w_sb[:, 1, 5:9], in_=w_d[:, 1, 5:9])
            nc.gpsimd.dma_start(out=xp0[:, 0, 42:58], in_=x_d[0, 0, :, 42:58])
            nc.gpsimd.dma_start(out=xp0[:, 1, 42:58], in_=x_d[0, 1, :, 42:58])

            def conv_group(xpad, xpadb, n, co, g0, gn, osb, stream_eng,
                           act_split=False):
                if act_split:
                    # Final group of the run: accumulate the two 4-row
                    # halves into separate PSUM tiles (same total
                    # streaming columns; LDWEIGHTS still hides under the
                    # two 224-col matmuls per tap) so the scalar ACT and
                    # the vector-engine affine can evacuate them in
                    # PARALLEL — the tile tracker serializes cross-engine
                    # readers of one PSUM tile. The halves' transfers
                    # then issue concurrently on the two HWDGE rings.
                    h = RG // 2
                    psA = pspool.tile([128, RG, W], f32, tag="ps")
                    psB = pspool.tile([128, RG, W], f32, tag="ps")
                    for t in range(NTAP):
                        kh, kw = divmod(t, 3)
                        for ps_, r0 in ((psA, g0 + kh), (psB, g0 + h + kh)):
                            if kw == 1 and xpadb is not None:
                                rhs = xpadb[:, :, r0:r0 + h, 0:W]
                            else:
                                rhs = xpad[:, :, r0:r0 + h, kw:kw + W]
                            nc.tensor.matmul(
                                out=ps_[:, 0:h],
                                lhsT=w_sb[:, co, t],
                                rhs=rhs,
                                start=(t == 0),
                                stop=(t == NTAP - 1),
                                perf_mode=DR,
                            )
                    ofin = opool.tile([128, RG - h, W], bf16, tag="ofin")
                    nc.scalar.activation(
                        out=osb[:, g0:g0 + h, :], in_=psA[:, 0:h],
                        func=mybir.ActivationFunctionType.Identity,
                        bias=sb_sb[:, NCO + co:NCO + co + 1],
                        scale=sb_sb[:, co:co + 1],
                    )
                    nc.vector.tensor_scalar(
                        ofin[:], psB[:, 0:h],
                        sb_sb[:, co:co + 1],
                        sb_sb[:, NCO + co:NCO + co + 1],
                        op0=mybir.AluOpType.mult,
                        op1=mybir.AluOpType.add,
                    )
                    dst = o_d[n, co * 128:(co + 1) * 128]
                    nc.scalar.dma_start(
                        out=dst[:, g0:g0 + h], in_=osb[:, g0:g0 + h]
                    )
                    nc.sync.dma_start(
                        out=dst[:, g0 + h:g0 + gn], in_=ofin[:],
                    )
                    return
                ps = pspool.tile([128, RG, W], f32, tag="ps")
                for t in range(NTAP):
                    kh, kw = divmod(t, 3)
                    if kw == 1 and xpadb is not None:
                        rhs = xpadb[:, :, g0 + kh:g0 + kh + gn, 0:W]
                    else:
                        rhs = xpad[:, :, g0 + kh:g0 + kh + gn, kw:kw + W]
                    nc.tensor.matmul(
                        out=ps[:, 0:gn],
                        lhsT=w_sb[:, co, t],
                        rhs=rhs,
                        start=(t == 0),
                        stop=(t == NTAP - 1),
                        perf_mode=DR,
                    )
                act = dict(
                    func=mybir.ActivationFunctionType.Identity,
                    bias=sb_sb[:, NCO + co:NCO + co + 1],
                    scale=sb_sb[:, co:co + 1],
                )
                nc.scalar.activation(
                    out=osb[:, g0:g0 + gn, :], in_=ps[:, 0:gn], **act,
                )
                if stream_eng is not None:
                    stream_eng.dma_start(
                        out=o_d[n, co * 128:(co + 1) * 128, g0:g0 + gn],
                        in_=osb[:, g0:g0 + gn],
                    )

            def load_image(n, eng, chunked=False):
                # Whole-plane DMAs per ci, then the idle vector engine
                # builds the 1-col-shifted B copy whose kw=1 windows start
                # even-aligned. chunked=True splits the transfers so DMA
                # completion-lane reuse self-paces them instead of
                # slamming ~1MB into the head-critical window.
                xp = xppool.tile([128, NCI, HP, WPP], fp8, tag="xpad")
                xpb = xpbpool.tile(
                    [128, NCI, HP, WPP], fp8, tag="xpadb", name="xpadb"
                )
                for ci in range(NCI):
                    if chunked:
                        for r0, r1 in ((0, 16), (16, 32), (32, 44), (44, 58)):
                            eng.dma_start(
                                out=xp[:, ci, r0:r1], in_=x_d[n, ci, :, r0:r1]
                            )
                    else:
                        eng.dma_start(out=xp[:, ci], in_=x_d[n, ci])
                for ci in range(NCI):
                    nc.vector.tensor_scalar(
                        xpb[:, ci, :, 0:56], xp[:, ci, :, 1:57],
                        -1.0, 1.0, **copy_ops,
                    )
                return xp, xpb

            staged = {}
            for n in range(NB):
                if n == 0:
                    xp, xpb = xp0, None
                elif n in staged:
                    xp, xpb = staged.pop(n)
                else:
                    # Images 2-3 load on gpsimd; the bufs=2 pools pace
                    # their transfers behind image n-2's last read.
                    xp, xpb = load_image(n, nc.gpsimd)
                osb = [
                    opool.tile([128, H, W], bf16, tag="osb", name=f"osb{i}")
                    for i in range(NCO)
                ]
                last_img = n == NB - 1
                if n == 0:
                    # Group-outer, co-inner, except co0 leads by four
                    # groups: each input chunk feeds PE work early
                    # (relaxed chunk deadlines) and the co1 weights get
                    # +6.8us. Image 1's loads ride the scalar queue
                    # between this image's ACTs, so their transfers can't
                    # steal bandwidth from the head-critical chunks.
                    order = [(0, 0), (1, 0), (2, 0), (3, 0), (0, 1), (1, 1),
                             (2, 1), (3, 1)]
                    order += [(c, co) for c in range(4, NCH)
                              for co in range(NCO)]
                    for u, (c, co) in enumerate(order):
                        conv_group(xp, xpb, n, co, c * RG, RG, osb[co],
                                   None)
                        if u == 3:
                            staged[1] = load_image(1, nc.scalar,
                                                   chunked=True)
                elif last_img:
                    # Stream every group as soon as its ACT lands, rotating
                    # three rings — but the two groups right before the
                    # final one go to the otherwise-idle gpsimd ring, so
                    # the final group's scalar+sync split transfers see no
                    # ring backlog.
                    rings = [nc.sync, nc.gpsimd, nc.scalar]
                    for co in range(NCO):
                        for c in range(NCH):
                            final = co == NCO - 1 and c == NCH - 1
                            if co == NCO - 1 and c >= NCH - 3 and not final:
                                ring = nc.gpsimd
                            else:
                                ring = rings[(co * NCH + c) % 3]
                            conv_group(
                                xp, xpb, n, co, c * RG, RG, osb[co],
                                None if final else ring,
                                act_split=final,
                            )
                else:
                    for co in range(NCO):
                        for c in range(NCH):
                            conv_group(xp, xpb, n, co, c * RG, RG, osb[co],
                                       None)
                if not last_img:
                    # bf16 result tiles: halves output HBM traffic (the
                    # e4m3 conv noise dwarfs the 2^-9 rounding).
                    for co in range(NCO):
                        dst = o_d[n, co * 128:(co + 1) * 128]
                        nc.sync.dma_start(out=dst[:, 0:36], in_=osb[co][:, 0:36])
                        nc.scalar.dma_start(out=dst[:, 36:H], in_=osb[co][:, 36:H])

    nc.compile()
    return nc


def _prep_weights(w_q, s, bias):
    # lhsT layout: [cin_k (128 partitions), co, tap, ci, cout_j] so that
    # w_t[k, co, t, ci, j] = w_q[co*128 + j, ci*128 + k, kh, kw]
    w_t = (
        np.asarray(w_q).astype(np.float32)
        .reshape(NCO, 128, NCI, 128, 3, 3)     # [co, j, ci, k, kh, kw]
        .transpose(3, 0, 4, 5, 2, 1)           # [k, co, kh, kw, ci, j]
        .reshape(128, NCO, NTAP, NCI, 128)
        .astype(mybir.dt.np(mybir.dt.float8e4))
    )
    sb_t = np.concatenate(
        [
            np.ascontiguousarray(
                np.asarray(s).reshape(NCO, 128).T.astype(np.float32)
            ),
            np.ascontiguousarray(
                np.asarray(bias).reshape(NCO, 128).T.astype(np.float32)
            ),
        ],
        axis=1,
    )
    return np.ascontiguousarray(w_t), np.ascontiguousarray(sb_t)


def _prep_x(x):
    # Host does the whole input transform: clamp, e4m3 quantize, zero-pad
    # into the conv-ready [N, NCI, 128, 58, 64] plane layout.
    fp8np = mybir.dt.np(mybir.dt.float8e4)
    xq = np.clip(np.asarray(x, dtype=np.float32), -1.0, 1.0).astype(fp8np)
    xp = np.zeros((N, NCI, 128, HP, WPP), dtype=fp8np)
    xp[:, :, :, 1:H + 1, 1:W + 1] = xq.reshape(N, NCI, 128, H, W)
    return xp


def kernel(x, w_q, s, bias):
    if "nc" not in _compiled:
        _compiled["nc"] = _build()
    nc = _compiled["nc"]

    w_t, sb_t = _prep_weights(w_q, s, bias)
    xp = _prep_x(x)
    core_ids = list(range(NCORES))
    in_maps = [
        {"x": xp[i * NB:(i + 1) * NB], "w": w_t, "sb": sb_t}
        for i in core_ids
    ]
    res = run_bass_kernel_spmd(nc, in_maps, core_ids)
    out = np.concatenate([res.results[i]["out"] for i in core_ids], axis=0)
    return out.astype(np.float32)


# revision 39
# speedup vs baseline: 1.0027x; 1.0027x over previous
"""BitConv2dInfer on 8 Trainium2 NeuronCores — fp8 DoubleRow, host-padded.

Reference computation (per full input):
    x = clip(x, -1, 1)                       # x [32, 256, 56, 56] f32
    y = conv2d(x, w_q, pad=1)                # w_q [256, 256, 3, 3] ternary
    y = y * s + bias                         # per-out-channel affine
Sharding: data-parallel over batch — each of the 8 cores gets 4 images and
the full (tiny) weights; outputs concatenate over batch with no comms.

Numerics: the clamp and the e4m3 quantization both happen on the HOST
(f32 -> clip -> e4m3), so the device sees conv-ready fp8 planes. Ternary
weights are exact in e4m3. The PE runs e4m3 DoubleRow matmuls (virtual
K=256, 2 fp8 mults/cell/cycle): 189.6 ns per 448-column matmul measured —
the DR roofline — and the steady-state stream is gapless.

Host also pre-pads: x ships as [NB, NCI, 128, 58, 64] zero-bordered planes
(64-wide rows keep the ci-plane stride 16B-aligned for the DoubleRow
ifmap AP rules; cols 58-63 never read). This kills the on-device clamp
and border memsets entirely and halves input HBM bytes vs bf16 transport.
The head of the kernel is ring-bandwidth-bound (~50-70 B/ns per ring
while all rings are hot, ~0.7us of descriptor-issue time per transfer),
so the first-matmul critical path carries only the head rows and the co0
weights. (Nibble-packing the ternary weights was tried and LOSES: the
DVE unpack latency on the critical path exceeds the ring-time saved.)

Device kernel (per core, per image):
  - image 0: row-chunked DMAs straight into the pad tile in deadline
    order on the two HWDGE rings (sync: ci0 rows + packed weights,
    scalar: ci1 rows; the slower SWDGE gpsimd ring gets only sb and the
    bottom rows); image 0 leads with four co0 groups so the co1 weights
    get +6.8us of slack
  - image 1 loads in row chunks on the scalar ring (DMA-lane reuse
    paces the chunks so they cannot crowd out the head-critical
    transfers); images 2-3 load whole planes on gpsimd, paced by the
    bufs=2 pad-tile pool (their transfers wait for image n-2's last
    read). The idle vector engine builds the 1-col-shifted copy B
    (even-aligned kw=1 tap) for images 1-3.
  - conv as 9 accumulated DoubleRow PE matmuls per (cout_tile, 8-row
    chunk): 189.6 ns each, stream measured gapless end to end
  - scalar-engine activation evacuates PSUM with per-partition scale+bias
  - images 0-2 DMA bf16 result tiles out over sync+scalar after each
    image; the last image streams each group as its ACT lands, rotating
    sync/gpsimd/scalar with the groups just before the final one forced
    to gpsimd, and the final group's transfer splits scalar+sync so the
    tail drains with no ring backlog

The PE clock gate (HAM) starts at 1.2 GHz and needs ~3.4us of sustained
activity to reach 2.4 GHz, so the kernel front-runs dummy matmuls on a
zeroed tile while the first input chunks are in flight.
"""

import sys

sys.path.insert(0, "/opt/trn_rl_repo")

import ml_dtypes  # noqa: F401
import numpy as np

import concourse.bass as bass  # noqa: F401  (registers engines)
import concourse.mybir as mybir
import concourse.tile as tile
from concourse import bacc
from concourse.bass_utils import run_bass_kernel_spmd

N, CIN, COUT, H, W = 32, 256, 256, 56, 56
NCORES = 8
NB = N // NCORES          # images per core
HP = H + 2                # padded rows
WPP = 64                  # padded row pitch (56+2 used; 64 keeps plane 16B-aligned)
RG = 8                    # output rows per PSUM chunk (8*56=448 <= 512 f32/bank)
NCH = H // RG             # chunks per image
NCI = CIN // 128          # cin tiles
NCO = COUT // 128         # cout tiles
NTAP = 9
N_WARM_MM = 10            # dummy matmuls to lift the HAM clock gate

_compiled = {}


def _build():
    nc = bacc.Bacc("TRN2", target_bir_lowering=False, debug=False)
    f32, bf16, fp8 = mybir.dt.float32, mybir.dt.bfloat16, mybir.dt.float8e4
    DR = mybir.MatmulPerfMode.DoubleRow
    x_d = nc.dram_tensor(
        "x", [NB, NCI, 128, HP, WPP], fp8, kind="ExternalInput"
    ).ap()
    w_d = nc.dram_tensor(
        "w", [128, NCO, NTAP, NCI, 128], fp8, kind="ExternalInput"
    ).ap()
    sb_d = nc.dram_tensor("sb", [128, 2 * NCO], f32, kind="ExternalInput").ap()
    o_d = nc.dram_tensor("out", [NB, COUT, H, W], bf16, kind="ExternalOutput").ap()

    # values are already in [-1,1], so the clamp is numerically a copy
    copy_ops = dict(op0=mybir.AluOpType.max, op1=mybir.AluOpType.min)

    with tile.TileContext(nc) as tc:
        with (
            tc.tile_pool(name="const", bufs=1) as cpool,
            # bufs=2 doubles as prefetch pacing: image n+2's DMA reuses
            # image n's buffer, so its transfer can't start (and steal
            # head/steady bandwidth) until image n's last matmul read.
            tc.tile_pool(name="xpad", bufs=2) as xppool,
            tc.tile_pool(name="xpadb", bufs=2) as xpbpool,
            tc.tile_pool(name="osb", bufs=3) as opool,
            tc.tile_pool(name="ps", bufs=6, space="PSUM") as pspool,
            tc.tile_pool(name="warmps", bufs=1, space="PSUM") as wpspool,
        ):
            w_sb = cpool.tile([128, NCO, NTAP, NCI, 128], fp8, tag="w")
            sb_sb = cpool.tile([128, 2 * NCO], f32, tag="sb")

            # HAM pre-warm: the memset rides the gpsimd queue, which clears
            # its preamble first, so the warm matmuls start the PE clock
            # ramp ~1us sooner than a vector-side memset would allow.
            # The product lands in a PSUM bank nobody reads.
            warm = cpool.tile([128, RG * W], fp8, tag="warm")
            nc.gpsimd.memset(warm[:], 0.0)
            warm_ps = wpspool.tile([128, RG * W], f32, tag="warmps")
            for _ in range(N_WARM_MM):
                nc.tensor.matmul(
                    out=warm_ps[:], lhsT=warm[:, 0:128], rhs=warm[:],
                    start=True, stop=True,
                )

            # First image: everything on the first-matmul critical path
            # rides the two HWDGE rings in deadline order (the SWDGE
            # gpsimd ring both issues and transfers much slower while the
            # HW rings are busy — it gets only sb and the bottom rows,
            # which aren't needed until ~28us). Weights split across both
            # rings; co1 has +6.8us of slack because image 0 leads with
            # four co0 groups.
            xp0 = xppool.tile([128, NCI, HP, WPP], fp8, tag="xpad")
            nc.sync.dma_start(out=xp0[:, 0, 0:10], in_=x_d[0, 0, :, 0:10])
            nc.scalar.dma_start(out=xp0[:, 1, 0:10], in_=x_d[0, 1, :, 0:10])
            # co0 weights in four tap pieces alternating rings, in tap
            # (deadline) order: the first-matmul gate is then only
            # max(ci0h+t0:3, ci1h+t3:5) instead of ci0h+all-of-co0.
            nc.sync.dma_start(out=w_sb[:, 0, 0:3], in_=w_d[:, 0, 0:3])
            nc.scalar.dma_start(out=w_sb[:, 0, 3:5], in_=w_d[:, 0, 3:5])
            nc.gpsimd.dma_start(out=sb_sb[:], in_=sb_d)
            nc.sync.dma_start(out=w_sb[:, 0, 5:7], in_=w_d[:, 0, 5:7])
            nc.scalar.dma_start(out=w_sb[:, 0, 7:9], in_=w_d[:, 0, 7:9])
            nc.gpsimd.dma_start(out=xp0[:, 0, 10:18], in_=x_d[0, 0, :, 10:18])
            nc.scalar.dma_start(out=xp0[:, 1, 10:26], in_=x_d[0, 1, :, 10:26])
            nc.sync.dma_start(out=xp0[:, 0, 18:26], in_=x_d[0, 0, :, 18:26])
            nc.gpsimd.dma_start(out=xp0[:, 0, 26:42], in_=x_d[0, 0, :, 26:42])
            nc.scalar.dma_start(out=xp0[:, 1, 26:42], in_=x_d[0, 1, :, 26:42])
            nc.sync.dma_start(out=w_sb[:, 1, 0:5], in_=w_d[:, 1, 0:5])
            nc.scalar.dma_start(out=w_sb[:, 1, 5:9], in_=w_d[:, 1, 5:9])
            nc.gpsimd.dma_start(out=xp0[:, 0, 42:58], in_=x_d[0, 0, :, 42:58])
            nc.gpsimd.dma_start(out=xp0[:, 1, 42:58], in_=x_d[0, 1, :, 42:58])

            def conv_group(xpad, xpadb, n, co, g0, gn, osb, stream_eng,
                           act_split=False):
                if act_split:
                    # Final group of the run: accumulate the two 4-row
                    # halves into separate PSUM tiles (same total
                    # streaming columns; LDWEIGHTS still hides under the
                    # two 224-col matmuls per tap) so the scalar ACT and
                    # the vector-engine affine can evacuate them in
                    # PARALLEL — the tile tracker serializes cross-engine
                    # readers of one PSUM tile. The halves' transfers
                    # then issue concurrently on the two HWDGE rings.
                    h = RG // 2
                    psA = pspool.tile([128, RG, W], f32, tag="ps")
                    psB = pspool.tile([128, RG, W], f32, tag="ps")
                    for t in range(NTAP):
                        kh, kw = divmod(t, 3)
                        for ps_, r0 in ((psA, g0 + kh), (psB, g0 + h + kh)):
                            if kw == 1 and xpadb is not None:
                                rhs = xpadb[:, :, r0:r0 + h, 0:W]
                            else:
                                rhs = xpad[:, :, r0:r0 + h, kw:kw + W]
                            nc.tensor.matmul(
                                out=ps_[:, 0:h],
                                lhsT=w_sb[:, co, t],
                                rhs=rhs,
                                start=(t == 0),
                                stop=(t == NTAP - 1),
                                perf_mode=DR,
                            )
                    ofin = opool.tile([128, RG - h, W], bf16, tag="ofin")
                    nc.scalar.activation(
                        out=osb[:, g0:g0 + h, :], in_=psA[:, 0:h],
                        func=mybir.ActivationFunctionType.Identity,
                        bias=sb_sb[:, NCO + co:NCO + co + 1],
                        scale=sb_sb[:, co:co + 1],
                    )
                    nc.vector.tensor_scalar(
                        ofin[:], psB[:, 0:h],
                        sb_sb[:, co:co + 1],
                        sb_sb[:, NCO + co:NCO + co + 1],
                        op0=mybir.AluOpType.mult,
                        op1=mybir.AluOpType.add,
                    )
                    dst = o_d[n, co * 128:(co + 1) * 128]
                    nc.scalar.dma_start(
                        out=dst[:, g0:g0 + h], in_=osb[:, g0:g0 + h]
                    )
                    nc.sync.dma_start(
                        out=dst[:, g0 + h:g0 + gn], in_=ofin[:],
                    )
                    return
                ps = pspool.tile([128, RG, W], f32, tag="ps")
                for t in range(NTAP):
                    kh, kw = divmod(t, 3)
                    if kw == 1 and xpadb is not None:
                        rhs = xpadb[:, :, g0 + kh:g0 + kh + gn, 0:W]
                    else:
                        rhs = xpad[:, :, g0 + kh:g0 + kh + gn, kw:kw + W]
                    nc.tensor.matmul(
                        out=ps[:, 0:gn],
                        lhsT=w_sb[:, co, t],
                        rhs=rhs,
                        start=(t == 0),
                        stop=(t == NTAP - 1),
                        perf_mode=DR,
                    )
                act = dict(
                    func=mybir.ActivationFunctionType.Identity,
                    bias=sb_sb[:, NCO + co:NCO + co + 1],
                    scale=sb_sb[:, co:co + 1],
                )
                nc.scalar.activation(
                    out=osb[:, g0:g0 + gn, :], in_=ps[:, 0:gn], **act,
                )
                if stream_eng is not None:
                    stream_eng.dma_start(
                        out=o_d[n, co * 128:(co + 1) * 128, g0:g0 + gn],
                        in_=osb[:, g0:g0 + gn],
                    )

            def load_image(n, eng, chunked=False):
                # Whole-plane DMAs per ci, then the idle vector engine
                # builds the 1-col-shifted B copy whose kw=1 windows start
                # even-aligned. chunked=True splits the transfers so DMA
                # completion-lane reuse self-paces them instead of
                # slamming ~1MB into the head-critical window.
                xp = xppool.tile([128, NCI, HP, WPP], fp8, tag="xpad")
                xpb = xpbpool.tile(
                    [128, NCI, HP, WPP], fp8, tag="xpadb", name="xpadb"
                )
                for ci in range(NCI):
                    if chunked:
                        for r0, r1 in ((0, 16), (16, 32), (32, 44), (44, 58)):
                            eng.dma_start(
                                out=xp[:, ci, r0:r1], in_=x_d[n, ci, :, r0:r1]
                            )
                    else:
                        eng.dma_start(out=xp[:, ci], in_=x_d[n, ci])
                for ci in range(NCI):
                    nc.vector.tensor_scalar(
                        xpb[:, ci, :, 0:56], xp[:, ci, :, 1:57],
                        -1.0, 1.0, **copy_ops,
                    )
                return xp, xpb

            staged = {}
            for n in range(NB):
                if n == 0:
                    xp, xpb = xp0, None
                elif n in staged:
                    xp, xpb = staged.pop(n)
                else:
                    # Images 2-3 load on gpsimd; the bufs=2 pools pace
                    # their transfers behind image n-2's last read.
                    xp, xpb = load_image(n, nc.gpsimd)
                osb = [
                    opool.tile([128, H, W], bf16, tag="osb", name=f"osb{i}")
                    for i in range(NCO)
                ]
                last_img = n == NB - 1
                if n == 0:
                    # Group-outer, co-inner, except co0 leads by four
                    # groups: each input chunk feeds PE work early
                    # (relaxed chunk deadlines) and the co1 weights get
                    # +6.8us. Image 1's loads ride the scalar queue
                    # between this image's ACTs, so their transfers can't
                    # steal bandwidth from the head-critical chunks.
                    order = [(0, 0), (1, 0), (2, 0), (3, 0), (0, 1), (1, 1),
                             (2, 1), (3, 1)]
                    order += [(c, co) for c in range(4, NCH)
                              for co in range(NCO)]
                    for u, (c, co) in enumerate(order):
                        conv_group(xp, xpb, n, co, c * RG, RG, osb[co],
                                   None)
                        if u == 3:
                            staged[1] = load_image(1, nc.scalar,
                                                   chunked=True)
                elif last_img:
                    # Stream every group as soon as its ACT lands, rotating
                    # three rings — but the two groups right before the
                    # final one go to the otherwise-idle gpsimd ring, so
                    # the final group's scalar+sync split transfers see no
                    # ring backlog.
                    rings = [nc.sync, nc.gpsimd, nc.scalar]
                    for co in range(NCO):
                        for c in range(NCH):
                            final = co == NCO - 1 and c == NCH - 1
                            if co == NCO - 1 and c >= NCH - 3 and not final:
                                ring = nc.gpsimd
                            else:
                                ring = rings[(co * NCH + c) % 3]
                            conv_group(
                                xp, xpb, n, co, c * RG, RG, osb[co],
                                None if final else ring,
                                act_split=final,
                            )
                else:
                    for co in range(NCO):
                        for c in range(NCH):
                            conv_group(xp, xpb, n, co, c * RG, RG, osb[co],
                                       None)
                if not last_img:
                    # bf16 result tiles: halves output HBM traffic (the
                    # e4m3 conv noise dwarfs the 2^-9 rounding).
                    for co in range(NCO):
                        dst = o_d[n, co * 128:(co + 1) * 128]
                        nc.sync.dma_start(out=dst[:, 0:36], in_=osb[co][:, 0:36])
                        nc.scalar.dma_start(out=dst[:, 36:H], in_=osb[co][:, 36:H])

    nc.compile()
    return nc


def _prep_weights(w_q, s, bias):
    # lhsT layout: [cin_k (128 partitions), co, tap, ci, cout_j] so that
    # w_t[k, co, t, ci, j] = w_q[co*128 + j, ci*128 + k, kh, kw]
    w_t = (
        np.asarray(w_q).astype(np.float32)
        .reshape(NCO, 128, NCI, 128, 3, 3)     # [co, j, ci, k, kh, kw]
        .transpose(3, 0, 4, 5, 2, 1)           # [k, co, kh, kw, ci, j]
        .reshape(128, NCO, NTAP, NCI, 128)
        .astype(mybir.dt.np(mybir.dt.float8e4))
    )
    sb_t = np.concatenate(
        [
            np.ascontiguousarray(
                np.asarray(s).reshape(NCO, 128).T.astype(np.float32)
            ),
            np.ascontiguousarray(
                np.asarray(bias).reshape(NCO, 128).T.astype(np.float32)
            ),
        ],
        axis=1,
    )
    return np.ascontiguousarray(w_t), np.ascontiguousarray(sb_t)


def _prep_x(x):
    # Host does the whole input transform: clamp, e4m3 quantize, zero-pad
    # into the conv-ready [N, NCI, 128, 58, 64] plane layout.
    fp8np = mybir.dt.np(mybir.dt.float8e4)
    xq = np.clip(np.asarray(x, dtype=np.float32), -1.0, 1.0).astype(fp8np)
    xp = np.zeros((N, NCI, 128, HP, WPP), dtype=fp8np)
    xp[:, :, :, 1:H + 1, 1:W + 1] = xq.reshape(N, NCI, 128, H, W)
    return xp


def kernel(x, w_q, s, bias):
    if "nc" not in _compiled:
        _compiled["nc"] = _build()
    nc = _compiled["nc"]

    w_t, sb_t = _prep_weights(w_q, s, bias)
    xp = _prep_x(x)
    core_ids = list(range(NCORES))
    in_maps = [
        {"x": xp[i * NB:(i + 1) * NB], "w": w_t, "sb": sb_t}
        for i in core_ids
    ]
    res = run_bass_kernel_spmd(nc, in_maps, core_ids)
    out = np.concatenate([res.results[i]["out"] for i in core_ids], axis=0)
    return out.astype(np.float32)
